# revision 46
# baseline (speedup 1.0000x reference)
"""Trainium2 Bass kernel for nn_AttentionLayer (pooling, dim=0 softmax).

Computation (full shapes B=64, T=2048, D=256):
    u = tanh(hs @ W^T + b)            [B,T,D]
    scores = u @ v                    [B,T]
    a = softmax(scores, axis=0)       (over the batch axis!)
    s[b] = a[b] @ hs[b]               [B,D]

Sharding: sequence-parallel over T across 8 cores (T_loc = 256). The
dim=0 softmax couples samples but not time steps, so each core's
softmax is fully local; only the final weighted sum needs a cross-core
reduction, done on the host (8 x 64KB partials).

The host pre-packs each core's shard into the two fp16 layouts the
device needs (natural [t, d] for the weighted sum, d-major for the
tanh matmul). Device HBM traffic is 2 x 8.4MB fp16 per core - the same
byte count as reading the f32 shard once, so this is roofline-neutral,
but it removes all on-device casts and DMA-transposes (whose
copy<->transpose mode switches serialize the DMA subsystem).

Per-core device pipeline (fp16 compute, f32 PSUM accumulation;
measured end-to-end rel err ~3e-3 vs the f32 reference):
  1. plain DMA loads of xt (d-major) and xn (natural) fp16 group tiles
  2. PE mm1: z^T[e, bt] = W-chunk @ Xt-chunk  (PSUM f32)
  3. ACT: u = tanh(z + bias)  PSUM -> SBUF fp16, per-partition bias
  4. PE mm2: scores rows [1, 512] = v^T @ u  (dense N=512 streams)
  5. DVE: scores -> fp16 row buffer; DRAM bounce; one DMA-transpose
     re-read lands scores in [t%128, bt//128] layout for the softmax
  6. softmax over b (free-dim stride-2 slices per t-chunk c); the
     normalized weights are written straight into a zero-initialized
     block-diagonal slab via a stride-(2B+1) access pattern
  7. PE step4: 128 matmuls lhsT=slab[:, j, :] (zero-padded a-columns)
     accumulate one [64, 256] PSUM tile in a single accumulation group
  8. one PSUM->SBUF copy + DMA out s_partial [64, 256] f32; host sums
     the 8 partials.
"""

import numpy as np

B, T, D = 64, 2048, 256
NCORES = 8
T_LOC = T // NCORES          # 256
BT = B * T_LOC               # 16384 rows per core
BTG = 1024                   # bt rows per pipeline group (2 PSUM banks)
PH = 128                     # partitions


def build_program(b_dim=B):
    """Build the per-core SPMD Bass program. b_dim scales the batch for
    cheap simulation (full run uses b_dim=64)."""
    import concourse.bacc as bacc
    import concourse.tile as tile
    from concourse import mybir

    F32 = mybir.dt.float32
    F16 = mybir.dt.float16
    AF = mybir.ActivationFunctionType
    AX = mybir.AxisListType

    bt = b_dim * T_LOC
    n_groups = bt // BTG
    tiles_per_g = BTG // PH   # 8
    n_cols = bt // PH         # scores columns (= b_dim*2)

    nc = bacc.Bacc("TRN2", target_bir_lowering=False, debug=False)

    # Host-prepacked fp16 inputs (see kernel() below):
    #   xn16[g, p, i, d]  = hs[g*1024 + i*128 + p, d]          (natural)
    #   xt16[g, p, m, q]  = hs[g*1024 + (m//2)*128 + q, (m%2)*128 + p]
    #   wt16[p, dc, ec, e'] = W[ec*128 + e', dc*128 + p]
    #   v16h[p, ec] = v[ec*128 + p]
    xn_d = nc.dram_tensor(
        "xn16", [n_groups, PH, tiles_per_g, D], F16, kind="ExternalInput"
    ).ap()
    xt_d = nc.dram_tensor(
        "xt16", [n_groups, PH, 2 * tiles_per_g, PH], F16, kind="ExternalInput"
    ).ap()
    wt_d = nc.dram_tensor("wt16", [PH, 2, 2, PH], F16, kind="ExternalInput").ap()
    bias_d = nc.dram_tensor("bias2", [PH, 2], F32, kind="ExternalInput").ap()
    v_d = nc.dram_tensor("v2", [PH, 2], F16, kind="ExternalInput").ap()
    out = nc.dram_tensor("out", [b_dim, D], F32, kind="ExternalOutput").ap()

    with tile.TileContext(nc) as tc:
        with (
            tc.tile_pool(name="singles", bufs=1) as singles,
            tc.tile_pool(name="xnat", bufs=n_groups) as xnat_pool,
            tc.tile_pool(name="xt", bufs=8) as xt_pool,
            tc.tile_pool(name="usb", bufs=6) as u_pool,
            tc.tile_pool(name="small", bufs=8) as small,
            tc.tile_pool(name="dram", bufs=1, space="DRAM") as dram_pool,
        ):
            # ---- constants (wt first: it gates the first matmul) ----
            wt = singles.tile([PH, 2, 2, PH], F16)
            nc.sync.dma_start(out=wt, in_=wt_d)
            bias_sb = singles.tile([PH, 2], F32)
            nc.gpsimd.dma_start(out=bias_sb, in_=bias_d)
            v16 = singles.tile([PH, 2], F16)
            nc.gpsimd.dma_start(out=v16, in_=v_d)

            # scores row buffer: scflat16[0, bt] = scores[bt] (fp16)
            scflat16 = singles.tile([1, bt], F16)
            # groups 0..g_split-1 bounce through DRAM + one xbar transpose
            # (overlapped with the loop); the last 2 groups' pieces are
    	    # staged at partition bases {0,32,64,96} and PE-transposed on
            # the idle PE, skipping the DRAM round-trip on the tail
            g_split = max(n_groups - 2, 0)
            if g_split > 0:
                scores_dram = dram_pool.tile([1, g_split * BTG], F16)
            else:
                scores_dram = None
            stage = singles.tile([PH, 512], F16)
            identity16 = singles.tile([PH, PH], F16)
            from concourse.masks import make_identity
            make_identity(nc, identity16)

            xnat_tiles = []

            with (
                tc.tile_pool(name="ups", bufs=3, space="PSUM") as ups_pool,
                tc.tile_pool(name="scps", bufs=2, space="PSUM") as scps_pool,
            ):
                for g in range(n_groups):
                    # ---- loads: xt alternates sync/scalar HWDGE queues,
                    #      xn on the gpsimd SWDGE queue ----
                    xt = xt_pool.tile([PH, 2 * tiles_per_g, PH], F16)
                    eng = nc.scalar if g % 2 == 0 else nc.sync
                    eng.dma_start(out=xt, in_=xt_d[g])
                    xn = xnat_pool.tile([PH, tiles_per_g, D], F16)
                    nc.gpsimd.dma_start(out=xn, in_=xn_d[g])
                    xnat_tiles.append(xn)

                    # ---- mm1 + tanh ----
                    u16 = []
                    for ec in range(2):
                        u_ps = ups_pool.tile([PH, BTG], F32)
                        for half in range(2):
                            for dc in range(2):
                                # rhs: tiles i in [half*4, half*4+4), m=i*2+dc
                                # half-outer order: 2 consecutive matmuls per
                                # PSUM bank (bank thrash triggers HAM dips)
                                m0 = half * 8 + dc
                                nc.tensor.matmul(
                                    u_ps[:, half * 512:(half + 1) * 512],
                                    wt[:, dc, ec, :],
                                    xt[:, m0:m0 + 7:2, :],
                                    start=(dc == 0),
                                    stop=(dc == 1),
                                )
                        u_sb = u_pool.tile([PH, BTG], F16)
                        nc.scalar.activation(
                            u_sb, u_ps, AF.Tanh, bias=bias_sb[:, ec:ec + 1]
                        )
                        u16.append(u_sb)

                    # ---- mm2: scores rows [1, 512] = v^T @ u ----
                    # ec-outer: consecutive matmuls share the v stationary
                    sc_list = []
                    for half in range(2):
                        sc_ps = scps_pool.tile([1, 512], F32)
                        sc_list.append(sc_ps)
                    for ec in range(2):
                        for half in range(2):
                            nc.tensor.matmul(
                                sc_list[half],
                                v16[:, ec:ec + 1],
                                u16[ec][:, half * 512:(half + 1) * 512],
                                start=(ec == 0),
                                stop=(ec == 1),
                            )
                    for half in range(2):
                        if g < g_split:
                            off = g * BTG + half * 512
                            nc.vector.tensor_copy(
                                scflat16[0:1, off:off + 512], sc_list[half]
                            )
                        else:
                            # split tail casts across DVE and the idle ACT
                            i_p = (g - g_split) * 2 + half
                            dst = stage[32 * i_p:32 * i_p + 1, :]
                            if i_p % 2 == 0:
                                nc.vector.tensor_copy(dst, sc_list[half])
                            else:
                                nc.scalar.copy(dst, sc_list[half])
                    # stream this group's scores to the DRAM bounce now so
                    # the final transpose read only waits on the last piece
                    if g < g_split:
                        nc.sync.dma_start(
                            out=scores_dram[0:1, g * BTG:(g + 1) * BTG],
                            in_=scflat16[0:1, g * BTG:(g + 1) * BTG],
                        )

            # ---- scatter scores to [q, j] layout via DRAM + xbar ----
            scmat16 = singles.tile([PH, n_cols], F16)
            rows_a = g_split * BTG // PH
            if scores_dram is not None:
                nc.sync.dma_start(
                    out=scmat16[:, 0:rows_a],
                    in_=scores_dram.rearrange("a (r q) -> (a r) q", q=PH),
                    transpose=True,
                )
            # tail: PE-transpose the staged pieces; stage[32i, k*128+q] =
            # scores[(rows_a + 4i + k)*128 + q], so transposed chunk k's
            # columns {0,32,64,96} are scmat columns rows_a+k :: 4
            with tc.tile_pool(name="tps", bufs=2, space="PSUM") as t_pool:
                for k in range(4):
                    t_ps = t_pool.tile([PH, PH], F16)
                    nc.tensor.transpose(
                        t_ps, stage[:, k * PH:(k + 1) * PH], identity16
                    )
                    nc.vector.tensor_copy(
                        scmat16[:, rows_a + k:n_cols:4],
                        t_ps[:, 0:97:32],
                    )

            # ---- softmax over b + write a into a block-diagonal slab ----
            # slab[q, j, b'] = a[b, c*128+q] if (j == b*2+c and b' == b) else 0
            # so lhsT = slab[:, j, :] is the zero-padded a-column for tile j
            # and all 128 step-4 matmuls accumulate one [b_dim, D] PSUM tile.
            slab = singles.tile([PH, n_cols, b_dim], F16)
            nc.vector.memset(slab, 0.0)
            sm_view = scmat16.rearrange("p (b c) -> p c b", c=2)
            # diagonal view: for fixed c, element b lands at free offset
            # (b*2+c)*b_dim + b  ->  stride b_dim*2+1, offset c*b_dim
            slab_flat = slab.rearrange("p j b -> p (j b)")
            for c in range(2):
                nm = small.tile([PH, 1], F32)
                nc.vector.reduce_max(
                    nm, sm_view[:, c, :], axis=AX.X, negate=True
                )
                e_sb = small.tile([PH, b_dim], F32)
                ssum = small.tile([PH, 1], F32)
                nc.scalar.activation(
                    e_sb, sm_view[:, c, :], AF.Exp, bias=nm, accum_out=ssum
                )
                rec = small.tile([PH, 1], F32)
                nc.vector.reciprocal(rec, ssum)
                st = 2 * b_dim + 1
                diag = slab_flat[
                    :, c * b_dim:c * b_dim + (b_dim - 1) * st + 1:st
                ]
                nc.vector.tensor_scalar_mul(diag, e_sb, rec)

            # ---- step4: one accumulation group over all 128 tiles ----
            s_sb = singles.tile([b_dim, D], F32)
            with tc.tile_pool(name="s4ps", bufs=1, space="PSUM") as s4_pool:
                s_ps = s4_pool.tile([b_dim, D], F32)
                for j in range(n_cols):
                    nc.tensor.matmul(
                        s_ps,
                        slab[:, j, :],
                        xnat_tiles[j // tiles_per_g][:, j % tiles_per_g, :],
                        start=(j == 0),
                        stop=(j == n_cols - 1),
                    )
                nc.vector.tensor_copy(s_sb, s_ps)

            nc.sync.dma_start(out=out, in_=s_sb)

    nc.compile()
    return nc


_prog_cache = {}


def _get_program(b_dim):
    if b_dim not in _prog_cache:
        _prog_cache[b_dim] = build_program(b_dim)
    return _prog_cache[b_dim]


def prep_core_inputs(shard_f32, w, bias, v):
    """Pack one core's [bt, D] f32 shard + weights into device layouts."""
    bt = shard_f32.shape[0]
    b_dim = bt // T_LOC
    n_groups = bt // BTG
    tiles_per_g = BTG // PH
    h16 = shard_f32.astype(np.float16)
    # xn16[g, p, i, d] = h16[g*1024 + i*128 + p, d]
    xn16 = np.ascontiguousarray(
        h16.reshape(n_groups, tiles_per_g, PH, D).transpose(0, 2, 1, 3)
    )
    # xt16[g, p, m, q] = h16[g*1024 + (m//2)*128 + q, (m%2)*128 + p]
    # h16 -> [g, i, q, dc, p] -> transpose to [g, p, (i, dc), q]
    hr = h16.reshape(n_groups, tiles_per_g, PH, 2, PH)
    xt16 = np.ascontiguousarray(hr.transpose(0, 4, 1, 3, 2)).reshape(
        n_groups, PH, 2 * tiles_per_g, PH
    )
    # wt16[p, dc, ec, e'] = W[ec*128 + e', dc*128 + p]
    w16 = w.astype(np.float16)
    wt16 = np.ascontiguousarray(
        w16.reshape(2, PH, 2, PH).transpose(3, 2, 0, 1)
    )
    bias2 = np.ascontiguousarray(bias.reshape(2, PH).T).astype(np.float32)
    v2 = np.ascontiguousarray(v.reshape(2, PH).T).astype(np.float16)
    return {
        "xn16": xn16,
        "xt16": xt16,
        "wt16": wt16,
        "bias2": bias2,
        "v2": v2,
    }


def kernel(hidden_states, W_attention, bias_attention, attention_vector):
    from concourse.bass_utils import run_bass_kernel_spmd

    hs = np.asarray(hidden_states, dtype=np.float32)
    w = np.asarray(W_attention, dtype=np.float32)
    bias = np.asarray(bias_attention, dtype=np.float32)
    v = np.asarray(attention_vector, dtype=np.float32)

    nc = _get_program(B)

    in_maps = []
    for core in range(NCORES):
        shard = np.ascontiguousarray(
            hs[:, core * T_LOC:(core + 1) * T_LOC, :]
        ).reshape(BT, D)
        in_maps.append(prep_core_inputs(shard, w, bias, v))

    res = run_bass_kernel_spmd(nc, in_maps, list(range(NCORES)))
    s = np.zeros((B, D), dtype=np.float32)
    for r in res.results:
        s += r["out"]
    return s


# revision 47
# speedup vs baseline: 1.0723x; 1.0723x over previous
"""Trainium2 Bass kernel for nn_AttentionLayer (pooling, dim=0 softmax).

Computation (full shapes B=64, T=2048, D=256):
    u = tanh(hs @ W^T + b)            [B,T,D]
    scores = u @ v                    [B,T]
    a = softmax(scores, axis=0)       (over the batch axis!)
    s[b] = a[b] @ hs[b]               [B,D]

Sharding: sequence-parallel over T across 8 cores (T_loc = 256). The
dim=0 softmax couples samples but not time steps, so each core's
softmax is fully local; only the final weighted sum needs a cross-core
reduction, done on the host (8 x 64KB partials).

The host pre-packs each core's shard into the two fp16 layouts the
device needs (natural [t, d] for the weighted sum, d-major for the
tanh matmul). Device HBM traffic is 2 x 8.4MB fp16 per core - the same
byte count as reading the f32 shard once, so this is roofline-neutral,
but it removes all on-device casts and DMA-transposes (whose
copy<->transpose mode switches serialize the DMA subsystem).

Per-core device pipeline (fp16 compute, f32 PSUM accumulation;
measured end-to-end rel err ~3e-3 vs the f32 reference):
  1. plain DMA loads of xt (d-major) and xn (natural) fp16 group tiles
  2. PE mm1: z^T[e, bt] = W-chunk @ Xt-chunk  (PSUM f32)
  3. ACT: u = tanh(z + bias)  PSUM -> SBUF fp16, per-partition bias
  4. PE mm2: scores rows [1, 512] = v^T @ u  (dense N=512 streams)
  5. DVE: scores -> fp16 row buffer; DRAM bounce; one DMA-transpose
     re-read lands scores in [t%128, bt//128] layout for the softmax
  6. softmax over b (free-dim stride-2 slices per t-chunk c); the
     normalized weights are written straight into a zero-initialized
     block-diagonal slab via a stride-(2B+1) access pattern
  7. PE step4: 128 matmuls lhsT=slab[:, j, :] (zero-padded a-columns)
     accumulate one [64, 256] PSUM tile in a single accumulation group
  8. one PSUM->SBUF copy + DMA out s_partial [64, 256] f32; host sums
     the 8 partials.
"""

import numpy as np

B, T, D = 64, 2048, 256
NCORES = 8
T_LOC = T // NCORES          # 256
BT = B * T_LOC               # 16384 rows per core
BTG = 1024                   # bt rows per pipeline group (2 PSUM banks)
PH = 128                     # partitions


def build_program(b_dim=B):
    """Build the per-core SPMD Bass program. b_dim scales the batch for
    cheap simulation (full run uses b_dim=64)."""
    import concourse.bacc as bacc
    import concourse.tile as tile
    from concourse import mybir

    F32 = mybir.dt.float32
    F16 = mybir.dt.float16
    AF = mybir.ActivationFunctionType
    AX = mybir.AxisListType

    bt = b_dim * T_LOC
    n_groups = bt // BTG
    tiles_per_g = BTG // PH   # 8
    n_cols = bt // PH         # scores columns (= b_dim*2)

    nc = bacc.Bacc("TRN2", target_bir_lowering=False, debug=False)

    # Host-prepacked fp16 inputs (see kernel() below):
    #   xn16[g, p, i, d]  = hs[g*1024 + i*128 + p, d]          (natural)
    #   xt16[g, p, m, q]  = hs[g*1024 + (m//2)*128 + q, (m%2)*128 + p]
    #   wt16[p, dc, ec, e'] = W[ec*128 + e', dc*128 + p]
    #   v16h[p, ec] = v[ec*128 + p]
    xn_d = nc.dram_tensor(
        "xn16", [n_groups, PH, tiles_per_g, D], F16, kind="ExternalInput"
    ).ap()
    xt_d = nc.dram_tensor(
        "xt16", [n_groups, PH, 2 * tiles_per_g, PH], F16, kind="ExternalInput"
    ).ap()
    wt_d = nc.dram_tensor("wt16", [PH, 2, 2, PH], F16, kind="ExternalInput").ap()
    bias_d = nc.dram_tensor("bias2", [PH, 2], F32, kind="ExternalInput").ap()
    v_d = nc.dram_tensor("v2", [PH, 2], F16, kind="ExternalInput").ap()
    out = nc.dram_tensor("out", [b_dim, D], F32, kind="ExternalOutput").ap()

    with tile.TileContext(nc) as tc:
        with (
            tc.tile_pool(name="singles", bufs=1) as singles,
            tc.tile_pool(name="xnat", bufs=n_groups) as xnat_pool,
            tc.tile_pool(name="xt", bufs=8) as xt_pool,
            tc.tile_pool(name="usb", bufs=4) as u_pool,
            tc.tile_pool(name="small", bufs=8) as small,
            tc.tile_pool(name="dram", bufs=1, space="DRAM") as dram_pool,
        ):
            # ---- constants (wt first: it gates the first matmul) ----
            wt = singles.tile([PH, 2, 2, PH], F16)
            nc.sync.dma_start(out=wt, in_=wt_d)
            bias_sb = singles.tile([PH, 2], F32)
            nc.gpsimd.dma_start(out=bias_sb, in_=bias_d)
            v16 = singles.tile([PH, 2], F16)
            nc.gpsimd.dma_start(out=v16, in_=v_d)

            # scores row buffer: scflat16[0, bt] = scores[bt] (fp16)
            scflat16 = singles.tile([1, bt], F16)
            # groups 0..g_split-1 bounce through DRAM + one xbar transpose
            # (overlapped with the loop); the last 2 groups' pieces are
    	    # staged at partition bases {0,32,64,96} and PE-transposed on
            # the idle PE, skipping the DRAM round-trip on the tail
            g_split = max(n_groups - 2, 0)
            if g_split > 0:
                scores_dram = dram_pool.tile([1, g_split * BTG], F16)
            else:
                scores_dram = None
            stage = singles.tile([PH, 512], F16)
            identity16 = singles.tile([PH, PH], F16)
            from concourse.masks import make_identity
            make_identity(nc, identity16)

            xnat_tiles = []

            with (
                tc.tile_pool(name="ups", bufs=3, space="PSUM") as ups_pool,
                tc.tile_pool(name="scps", bufs=2, space="PSUM") as scps_pool,
            ):
                for g in range(n_groups):
                    # ---- loads: xt alternates sync/scalar HWDGE queues,
                    #      xn on the gpsimd SWDGE queue ----
                    xt = xt_pool.tile([PH, 2 * tiles_per_g, PH], F16)
                    eng = nc.scalar if g % 2 == 0 else nc.sync
                    eng.dma_start(out=xt, in_=xt_d[g])
                    xn = xnat_pool.tile([PH, tiles_per_g, D], F16)
                    nc.gpsimd.dma_start(out=xn, in_=xn_d[g])
                    xnat_tiles.append(xn)

                    # ---- mm1 + tanh ----
                    u16 = []
                    for ec in range(2):
                        u_ps = ups_pool.tile([PH, BTG], F32)
                        for half in range(2):
                            for dc in range(2):
                                # rhs: tiles i in [half*4, half*4+4), m=i*2+dc
                                # half-outer order: 2 consecutive matmuls per
                                # PSUM bank (bank thrash triggers HAM dips)
                                m0 = half * 8 + dc
                                nc.tensor.matmul(
                                    u_ps[:, half * 512:(half + 1) * 512],
                                    wt[:, dc, ec, :],
                                    xt[:, m0:m0 + 7:2, :],
                                    start=(dc == 0),
                                    stop=(dc == 1),
                                )
                        u_sb = u_pool.tile([PH, BTG], F16)
                        nc.scalar.activation(
                            u_sb, u_ps, AF.Tanh, bias=bias_sb[:, ec:ec + 1]
                        )
                        u16.append(u_sb)

                    # ---- mm2: scores rows [1, 512] = v^T @ u ----
                    # ec-outer: consecutive matmuls share the v stationary
                    sc_list = []
                    for half in range(2):
                        sc_ps = scps_pool.tile([1, 512], F32)
                        sc_list.append(sc_ps)
                    for ec in range(2):
                        for half in range(2):
                            nc.tensor.matmul(
                                sc_list[half],
                                v16[:, ec:ec + 1],
                                u16[ec][:, half * 512:(half + 1) * 512],
                                start=(ec == 0),
                                stop=(ec == 1),
                            )
                    for half in range(2):
                        if g < g_split:
                            off = g * BTG + half * 512
                            nc.vector.tensor_copy(
                                scflat16[0:1, off:off + 512], sc_list[half]
                            )
                        else:
                            # split tail casts across DVE and the idle ACT
                            i_p = (g - g_split) * 2 + half
                            dst = stage[32 * i_p:32 * i_p + 1, :]
                            if i_p % 2 == 0:
                                nc.vector.tensor_copy(dst, sc_list[half])
                            else:
                                nc.scalar.copy(dst, sc_list[half])
                    # stream this group's scores to the DRAM bounce now so
                    # the final transpose read only waits on the last piece
                    if g < g_split:
                        nc.sync.dma_start(
                            out=scores_dram[0:1, g * BTG:(g + 1) * BTG],
                            in_=scflat16[0:1, g * BTG:(g + 1) * BTG],
                        )

            # ---- scatter scores to [q, j] layout via DRAM + xbar ----
            scmat16 = singles.tile([PH, n_cols], F16)
            rows_a = g_split * BTG // PH
            if scores_dram is not None:
                nc.sync.dma_start(
                    out=scmat16[:, 0:rows_a],
                    in_=scores_dram.rearrange("a (r q) -> (a r) q", q=PH),
                    transpose=True,
                )
            # tail: PE-transpose the staged pieces; stage[32i, k*128+q] =
            # scores[(rows_a + 4i + k)*128 + q], so transposed chunk k's
            # columns {0,32,64,96} are scmat columns rows_a+k :: 4
            with tc.tile_pool(name="tps", bufs=2, space="PSUM") as t_pool:
                for k in range(4):
                    t_ps = t_pool.tile([PH, PH], F16)
                    nc.tensor.transpose(
                        t_ps, stage[:, k * PH:(k + 1) * PH], identity16
                    )
                    nc.vector.tensor_copy(
                        scmat16[:, rows_a + k:n_cols:4],
                        t_ps[:, 0:97:32],
                    )

            # ---- softmax over b + write a into a block-diagonal slab ----
            # slab[q, j, b'] = a[b, c*128+q] if (j == b*2+c and b' == b) else 0
            # so lhsT = slab[:, j, :] is the zero-padded a-column for tile j
            # and all 128 step-4 matmuls accumulate one [b_dim, D] PSUM tile.
            slab = singles.tile([PH, n_cols, b_dim], F16)
            nc.vector.memset(slab, 0.0)
            sm_view = scmat16.rearrange("p (b c) -> p c b", c=2)
            # diagonal view: for fixed c, element b lands at free offset
            # (b*2+c)*b_dim + b  ->  stride b_dim*2+1, offset c*b_dim
            slab_flat = slab.rearrange("p j b -> p (j b)")
            for c in range(2):
                nm = small.tile([PH, 1], F32)
                nc.vector.reduce_max(
                    nm, sm_view[:, c, :], axis=AX.X, negate=True
                )
                e_sb = small.tile([PH, b_dim], F32)
                ssum = small.tile([PH, 1], F32)
                nc.scalar.activation(
                    e_sb, sm_view[:, c, :], AF.Exp, bias=nm, accum_out=ssum
                )
                rec = small.tile([PH, 1], F32)
                nc.vector.reciprocal(rec, ssum)
                st = 2 * b_dim + 1
                diag = slab_flat[
                    :, c * b_dim:c * b_dim + (b_dim - 1) * st + 1:st
                ]
                nc.vector.tensor_scalar_mul(diag, e_sb, rec)

            # ---- step4: one accumulation group over all 128 tiles ----
            s_sb = singles.tile([b_dim, D], F32)
            with tc.tile_pool(name="s4ps", bufs=1, space="PSUM") as s4_pool:
                s_ps = s4_pool.tile([b_dim, D], F32)
                for j in range(n_cols):
                    nc.tensor.matmul(
                        s_ps,
                        slab[:, j, :],
                        xnat_tiles[j // tiles_per_g][:, j % tiles_per_g, :],
                        start=(j == 0),
                        stop=(j == n_cols - 1),
                    )
                nc.vector.tensor_copy(s_sb, s_ps)

            nc.sync.dma_start(out=out, in_=s_sb)

    nc.compile()
    return nc


_prog_cache = {}


def _get_program(b_dim):
    if b_dim not in _prog_cache:
        _prog_cache[b_dim] = build_program(b_dim)
    return _prog_cache[b_dim]


def prep_core_inputs(shard_f32, w, bias, v):
    """Pack one core's [bt, D] f32 shard + weights into device layouts."""
    bt = shard_f32.shape[0]
    b_dim = bt // T_LOC
    n_groups = bt // BTG
    tiles_per_g = BTG // PH
    h16 = shard_f32.astype(np.float16)
    # xn16[g, p, i, d] = h16[g*1024 + i*128 + p, d]
    xn16 = np.ascontiguousarray(
        h16.reshape(n_groups, tiles_per_g, PH, D).transpose(0, 2, 1, 3)
    )
    # xt16[g, p, m, q] = h16[g*1024 + (m//2)*128 + q, (m%2)*128 + p]
    # h16 -> [g, i, q, dc, p] -> transpose to [g, p, (i, dc), q]
    hr = h16.reshape(n_groups, tiles_per_g, PH, 2, PH)
    xt16 = np.ascontiguousarray(hr.transpose(0, 4, 1, 3, 2)).reshape(
        n_groups, PH, 2 * tiles_per_g, PH
    )
    # wt16[p, dc, ec, e'] = W[ec*128 + e', dc*128 + p]
    w16 = w.astype(np.float16)
    wt16 = np.ascontiguousarray(
        w16.reshape(2, PH, 2, PH).transpose(3, 2, 0, 1)
    )
    bias2 = np.ascontiguousarray(bias.reshape(2, PH).T).astype(np.float32)
    v2 = np.ascontiguousarray(v.reshape(2, PH).T).astype(np.float16)
    return {
        "xn16": xn16,
        "xt16": xt16,
        "wt16": wt16,
        "bias2": bias2,
        "v2": v2,
    }


def kernel(hidden_states, W_attention, bias_attention, attention_vector):
    from concourse.bass_utils import run_bass_kernel_spmd

    hs = np.asarray(hidden_states, dtype=np.float32)
    w = np.asarray(W_attention, dtype=np.float32)
    bias = np.asarray(bias_attention, dtype=np.float32)
    v = np.asarray(attention_vector, dtype=np.float32)

    nc = _get_program(B)

    in_maps = []
    for core in range(NCORES):
        shard = np.ascontiguousarray(
            hs[:, core * T_LOC:(core + 1) * T_LOC, :]
        ).reshape(BT, D)
        in_maps.append(prep_core_inputs(shard, w, bias, v))

    res = run_bass_kernel_spmd(nc, in_maps, list(range(NCORES)))
    s = np.zeros((B, D), dtype=np.float32)
    for r in res.results:
        s += r["out"]
    return s
